# revision 3
# baseline (speedup 1.0000x reference)
"""CRF negative log-likelihood on 8 Trainium2 NeuronCores.

Strategy (v2)
-------------
The dominant cost is the forward algorithm (log-partition): a length-T
recurrence of "log-matmuls"  alpha_t = em_t + LSE_i(alpha_{t-1} + trans).
In exp-domain this is  u_t = exp(em_t - c) * (expT^T @ u_{t-1}), i.e. a
128x128 matmul + elementwise multiply per step.

transitions are in [-0.1, 0.1], so exp(trans) is a strong Hilbert-metric
contraction (~0.1 per step): the recurrence forgets its initial condition
in a few steps.  We split T into C chunks per core, warm each chunk up W
steps early from a ones-vector, and run chunks in lockstep as columns of
G=2 interleaved state blocks so PE (matmul) and DVE/ACT (evacuate +
emission multiply) pipeline instead of ping-ponging.

All recurrence data is bf16 (PE runs 4x faster than fp32; DVE gets 2x
packed mode).  exp(em - c) is precomputed on the host and preloaded to
SBUF in large DMAs.  PSUM evacuation (fp32, the TRN2 bottleneck: one
32-bit PSUM read port per lane on DVE and ACT each) is split: ACT
copies+casts a fraction XS of columns to bf16 (DVE then multiplies those
at 2x), DVE multiplies the rest directly from PSUM at 1x.

Per-chunk log-gains are recovered from boundary column-sums (ones /
exp(end) matmuls, fp32) and telescoped into log_Z on the host in f64.
The gold-path score (pure gathers, ~0.006% of FLOPs) and the final mean
are computed on the host.

Sharding: data-parallel over batch B: core i owns b in [32*i, 32*i+32).
"""

import numpy as np
import ml_dtypes
from contextlib import ExitStack

import concourse.bass as bass
import concourse.tile as tile
from concourse import bacc, mybir
from concourse.bass_utils import run_bass_kernel_spmd

# Problem shape (hardcoded per harness contract).
B, T, K = 256, 1024, 128
N_CORES = 8
BC = B // N_CORES          # 32 batch rows per core

# Tunables.
C = 32                     # time chunks per core (must divide T; G | C)
G = 2                      # interleaved pipeline groups
W = 4                      # warmup steps per chunk
XS = 424                   # columns evacuated via ACT copy (rest: DVE 1x)
PRE = 4                    # preload DMA granularity (virtual steps per DMA)

TC = T // C                # steps per chunk
NV = TC + W - 1            # matmul virtual-steps
CG = C // G                # chunks per group
COLS_G = CG * BC           # state columns per group
CSHIFT = float(np.log(128.0) + 0.5)  # per-step rescale (exactness-neutral)

BF16 = mybir.dt.bfloat16
F32 = mybir.dt.float32

_NC_CACHE = None


def _build_program(repeat=1):
    """Build the per-core SPMD Bass program (identical on all cores).

    repeat > 1 wraps the whole computation in an on-device loop - used
    only by the test harness for differential HW timing.
    """
    nc = bacc.Bacc("TRN2", target_bir_lowering=False, debug=False,
                   num_devices=N_CORES)

    tot_cols = NV * G * COLS_G
    emx = nc.dram_tensor("emx", [K, tot_cols], BF16, kind="ExternalInput").ap()
    expt_d = nc.dram_tensor("expt", [K, K], BF16, kind="ExternalInput").ap()
    onesend_d = nc.dram_tensor("onesend", [K, 2], BF16,
                               kind="ExternalInput").ap()
    startexp_d = nc.dram_tensor("startexp", [K, 1], F32,
                                kind="ExternalInput").ap()
    sums = nc.dram_tensor("sums", [2, 2 * G * COLS_G], F32,
                          kind="ExternalOutput").ap()

    with tile.TileContext(nc) as tc, ExitStack() as ctx:
        const_pool = ctx.enter_context(tc.tile_pool(name="const", bufs=1))
        state_pool = ctx.enter_context(tc.tile_pool(name="state", bufs=4))
        mid_pool = ctx.enter_context(tc.tile_pool(name="mid", bufs=4))
        psum_pool = ctx.enter_context(
            tc.tile_pool(name="psum", bufs=4, space="PSUM"))
        bsum_pool = ctx.enter_context(
            tc.tile_pool(name="bsum", bufs=2, space="PSUM"))

        # Constants (loaded once, outside the repeat loop).
        expT = const_pool.tile([K, K], BF16)
        nc.sync.dma_start(expT[:], expt_d[:])
        onesend = const_pool.tile([K, 2], BF16)
        nc.sync.dma_start(onesend[:], onesend_d[:])
        startexp = const_pool.tile([K, 1], F32)
        nc.sync.dma_start(startexp[:], startexp_d[:])

        e_sb = const_pool.tile([K, tot_cols], BF16)
        out_sb = const_pool.tile([2, 2 * G * COLS_G], F32)

        loop_cm = tc.For_i(0, repeat, 1) if repeat > 1 else None
        if loop_cm is not None:
            ctx.enter_context(loop_cm)

        # Preload all emissions to SBUF in large DMAs.
        step_cols = G * COLS_G
        pre_cols = PRE * step_cols
        for c0 in range(0, tot_cols, pre_cols):
            c1 = min(tot_cols, c0 + pre_cols)
            nc.sync.dma_start(e_sb[:, c0:c1], emx[:, c0:c1])

        v = []
        for g in range(G):
            vg = state_pool.tile([K, COLS_G], BF16)
            nc.vector.memset(vg[:], 1.0)
            v.append(vg)

        for s in range(1, NV + 1):
            for g in range(G):
                ps = psum_pool.tile([K, COLS_G], F32)
                for n0 in range(0, COLS_G, 512):
                    n1 = min(COLS_G, n0 + 512)
                    nc.tensor.matmul(ps[:, n0:n1], expT[:], v[g][:, n0:n1],
                                     start=True, stop=True)

                ecol = ((s - 1) * G + g) * COLS_G
                e_t = e_sb[:, ecol:ecol + COLS_G]

                vn = state_pool.tile([K, COLS_G], BF16)
                # ACT path: copy+cast cols [0:XS] to bf16, DVE multiplies
                # them at 2x; DVE path: cols [XS:] at 1x from PSUM.
                if XS > 0:
                    mid = mid_pool.tile([K, XS], BF16)
                    nc.scalar.copy(mid[:], ps[:, 0:XS])
                    nc.vector.tensor_mul(vn[:, 0:XS], mid[:], e_t[:, 0:XS])
                if XS < COLS_G:
                    nc.vector.tensor_mul(vn[:, XS:], ps[:, XS:], e_t[:, XS:])
                v[g] = vn

                if s == W:
                    if g == 0:
                        # chunk 0 exact init at t=0:
                        # u0 = exp(start) * exp(em0 - c)
                        nc.vector.tensor_scalar_mul(v[0][:, 0:BC],
                                                    e_t[:, 0:BC],
                                                    startexp[:])
                if s == W - 1:
                    # entry boundary sums: 1^T v (state time = c*TC - 1)
                    bp = bsum_pool.tile([1, COLS_G], F32)
                    for n0 in range(0, COLS_G, 512):
                        n1 = min(COLS_G, n0 + 512)
                        nc.tensor.matmul(bp[:, n0:n1], onesend[:, 0:1],
                                         v[g][:, n0:n1], start=True,
                                         stop=True)
                    nc.scalar.copy(out_sb[0:1, g * COLS_G:(g + 1) * COLS_G],
                                   bp[:])

        # final boundary sums: [1^T v ; exp(end)^T v]
        for g in range(G):
            bp = bsum_pool.tile([2, COLS_G], F32)
            for n0 in range(0, COLS_G, 512):
                n1 = min(COLS_G, n0 + 512)
                nc.tensor.matmul(bp[:, n0:n1], onesend[:], v[g][:, n0:n1],
                                 start=True, stop=True)
            off = G * COLS_G + g * COLS_G
            nc.scalar.copy(out_sb[0:2, off:off + COLS_G], bp[:])

        nc.sync.dma_start(sums[:], out_sb[:])

    nc.compile()
    return nc


def _host_prep(emissions, transitions, start_transitions, end_transitions):
    """Per-core input maps with host-precomputed exp and chunk layout.

    emx[k, ((s-1)*G + g)*COLS_G + l*BC + b]
      = exp(em[core*BC + b, tau, k] - CSHIFT),
      tau = clip((g*CG + l)*TC - W + s, 0, T-1).
    """
    e_full = np.exp(emissions.astype(np.float32) - CSHIFT)
    e_full = e_full.astype(ml_dtypes.bfloat16)

    s_idx = np.arange(1, NV + 1)
    g_idx = np.arange(G)
    l_idx = np.arange(CG)
    # tau[s, g, l]
    tau = (g_idx[None, :, None] * CG + l_idx[None, None, :]) * TC \
        - W + s_idx[:, None, None]
    tau = np.clip(tau, 0, T - 1)

    expt_in = np.ascontiguousarray(
        np.exp(transitions.astype(np.float32)).astype(ml_dtypes.bfloat16))
    onesend_in = np.stack(
        [np.ones(K, np.float32),
         np.exp(end_transitions.astype(np.float32))], axis=1)
    onesend_in = np.ascontiguousarray(onesend_in.astype(ml_dtypes.bfloat16))
    startexp_in = np.ascontiguousarray(
        np.exp(start_transitions.astype(np.float32))[:, None])

    in_maps = []
    for core in range(N_CORES):
        ec = e_full[core * BC:(core + 1) * BC]              # [BC, T, K]
        eT = np.ascontiguousarray(ec.transpose(2, 1, 0))    # [K, T, BC]
        emx = eT[:, tau, :].reshape(K, NV * G * COLS_G)
        in_maps.append({
            "emx": np.ascontiguousarray(emx),
            "expt": expt_in,
            "onesend": onesend_in,
            "startexp": startexp_in,
        })
    return in_maps


def _gold_score(em, tags, mask, trans, start, end):
    em = em.astype(np.float64)
    mask = mask.astype(np.float64)
    tg = tags.astype(np.int64)
    score = start.astype(np.float64)[tg[:, 0]]
    emit = np.take_along_axis(em, tg[:, :, None], axis=2)[:, :, 0]
    score = score + (emit * mask).sum(axis=1)
    score = score + (trans.astype(np.float64)[tg[:, :-1], tg[:, 1:]]
                     * mask[:, 1:]).sum(axis=1)
    seq_ends = mask.astype(np.int64).sum(axis=1) - 1
    last = tg[np.arange(tg.shape[0]), seq_ends]
    score = score + end.astype(np.float64)[last]
    return score


def _host_logz_fallback(em, trans, start, end):
    """Exact f64 forward algorithm (only used if mask is not all-ones)."""
    em = em.astype(np.float64)
    la = start.astype(np.float64) + em[:, 0, :]
    tr = trans.astype(np.float64)
    for t in range(1, em.shape[1]):
        sc = tr[None] + la[:, :, None] + em[:, t, None, :]
        m = sc.max(axis=1, keepdims=True)
        la = np.squeeze(m, 1) + np.log(np.exp(sc - m).sum(axis=1))
    x = la + end[None].astype(np.float64)
    m = x.max(axis=1, keepdims=True)
    return np.squeeze(m, 1) + np.log(np.exp(x - m).sum(axis=1))


def _assemble_logz(results):
    """Telescoped per-chunk log-gains -> logz[B] (host, f64)."""
    logz = np.zeros(B)
    for core in range(N_CORES):
        r = np.asarray(results[core]["sums"], dtype=np.float64)
        # per global chunk c: group g = c // CG, local l = c % CG
        acc = None
        for c in range(C):
            g, l = divmod(c, CG)
            sl = slice(g * COLS_G + l * BC, g * COLS_G + l * BC + BC)
            entry = r[0, sl]
            off = G * COLS_G
            sl2 = slice(off + g * COLS_G + l * BC,
                        off + g * COLS_G + l * BC + BC)
            end1 = r[0, sl2]
            endE = r[1, sl2]
            if c == 0:
                acc = np.log(end1).copy()         # exact absolute scale
            elif c < C - 1:
                acc += np.log(end1) - np.log(entry)
            else:
                acc += np.log(endE) - np.log(entry)
        logz[core * BC:(core + 1) * BC] = acc + T * CSHIFT
    return logz


def kernel(emissions, tags, mask, transitions, start_transitions,
           end_transitions):
    global _NC_CACHE
    emissions = np.ascontiguousarray(np.asarray(emissions, dtype=np.float32))
    tags = np.asarray(tags)
    mask = np.asarray(mask)
    transitions = np.asarray(transitions, dtype=np.float32)
    start_transitions = np.asarray(start_transitions, dtype=np.float32)
    end_transitions = np.asarray(end_transitions, dtype=np.float32)

    score = _gold_score(emissions, tags, mask, transitions,
                        start_transitions, end_transitions)

    if not np.all(mask == 1):
        logz = _host_logz_fallback(emissions, transitions,
                                   start_transitions, end_transitions)
        return np.float32(-(score - logz).mean())

    if _NC_CACHE is None:
        _NC_CACHE = _build_program()
    nc = _NC_CACHE

    in_maps = _host_prep(emissions, transitions, start_transitions,
                         end_transitions)
    results = run_bass_kernel_spmd(nc, in_maps, list(range(N_CORES))).results
    logz = _assemble_logz(results)
    return np.float32(-(score - logz).mean())


# revision 11
# speedup vs baseline: 14.6585x; 14.6585x over previous
"""CRF negative log-likelihood on 8 Trainium2 NeuronCores.

Strategy (v3)
-------------
The dominant cost is the forward algorithm (log-partition): a length-T
recurrence of "log-matmuls"  alpha_t = em_t + LSE_i(alpha_{t-1} + trans).
In exp-domain this is  u_t = exp(em_t - c) * (expT^T @ u_{t-1}), i.e. a
128x128 matmul + elementwise multiply per step.

transitions are in [-0.1, 0.1], so exp(trans) is a strong Hilbert-metric
contraction (~0.1 per step): the recurrence forgets its initial condition
in a few steps.  We split T into C chunks per core, warm each chunk up W
steps early from a ones-vector, and run chunks in lockstep as columns of
G interleaved state blocks so PE (matmul) and DVE/ACT (evacuate +
emission multiply) pipeline instead of ping-ponging.

All recurrence data is bf16 (PE runs 4x faster than fp32; DVE gets 2x
packed mode).  exp(em - c) is precomputed on the host and preloaded to
SBUF in one large DMA (per-dma_start fixed cost here is ~50us, so DMA
count matters far more than bytes).  PSUM evacuation (fp32: one 32-bit
PSUM read port per lane on DVE / ACT) is split: ACT copies+casts XS
columns to bf16 (DVE then multiplies those at 2x), DVE multiplies the
rest directly from PSUM at 1x.  Boundary states are snapshotted with one
cheap bf16 copy and ALL boundary column-sum matmuls run after the loop,
off the recurrence critical path.

Per-chunk log-gains (boundary sums) telescope into log_Z on the host in
f64.  The gold-path score and final mean are host-side.

Sharding: data-parallel over batch B: core i owns b in [32*i, 32*i+32).
"""

import numpy as np
import ml_dtypes
from contextlib import ExitStack

import concourse.bass as bass
import concourse.tile as tile
from concourse import bacc, mybir
from concourse.bass_utils import run_bass_kernel_spmd

# Problem shape (hardcoded per harness contract).
B, T, K = 256, 1024, 128
N_CORES = 8
BC = B // N_CORES          # 32 batch rows per core

# Tunables.
C = 32                     # time chunks per core (must divide T; G | C)
G = 2                      # interleaved pipeline groups
W = 3                      # warmup steps per chunk
XS = 0                     # columns evacuated via ACT copy (rest: DVE 1x)
PRE = 9                    # preload DMA granularity (virtual steps per DMA)
DMA_GPSIMD = True          # issue preload/out DMAs on gpsimd (SWDGE)
E8 = True                  # emissions in fp8e4 (e4m3); else bf16
C_E = 1.0                  # emission exp shift when E8 (rest folded into M)
PRELOAD_IN_LOOP = True     # False: hoist preload out of the repeat loop
                           # (diagnostic only - isolates compute cost)

TC = T // C                # steps per chunk
NV = TC + W - 1            # matmul virtual-steps
CG = C // G                # chunks per group
COLS_G = CG * BC           # state columns per group
CSHIFT = float(np.log(128.0) + 0.5)  # per-step rescale (exactness-neutral)

BF16 = mybir.dt.bfloat16
F32 = mybir.dt.float32
FP8 = mybir.dt.float8e4

_NC_CACHE = None


def _build_program(repeat=1):
    """Build the per-core SPMD Bass program (identical on all cores).

    repeat > 1 wraps the whole computation in an on-device loop - used
    only by the test harness for differential HW timing.
    """
    nc = bacc.Bacc("TRN2", target_bir_lowering=False, debug=False,
                   num_devices=N_CORES)

    tot_cols = NV * G * COLS_G
    e_dt = FP8 if E8 else BF16
    emx = nc.dram_tensor("emx", [K, tot_cols], e_dt, kind="ExternalInput").ap()
    expt_d = nc.dram_tensor("expt", [K, K], BF16, kind="ExternalInput").ap()
    onesend_d = nc.dram_tensor("onesend", [K, 2], BF16,
                               kind="ExternalInput").ap()
    startexp_d = nc.dram_tensor("startexp", [K, 1], F32,
                                kind="ExternalInput").ap()
    sums = nc.dram_tensor("sums", [2, 2 * G * COLS_G], F32,
                          kind="ExternalOutput").ap()

    with tile.TileContext(nc) as tc, ExitStack() as ctx:
        const_pool = ctx.enter_context(tc.tile_pool(name="const", bufs=1))
        state_pool = ctx.enter_context(tc.tile_pool(name="state", bufs=2 * G))
        mid_pool = ctx.enter_context(tc.tile_pool(name="mid", bufs=4))
        psum_pool = ctx.enter_context(
            tc.tile_pool(name="psum", bufs=max(2, 6 // max(1, (4 * COLS_G) // 2048)),
                         space="PSUM"))
        bsum_pool = ctx.enter_context(
            tc.tile_pool(name="bsum", bufs=1, space="PSUM"))

        # Constants (loaded once, outside the repeat loop; SWDGE - the
        # sync-queue HWDGE path costs ~50-95us per dma_start here).
        expT = const_pool.tile([K, K], BF16)
        nc.gpsimd.dma_start(expT[:], expt_d[:])
        onesend = const_pool.tile([K, 2], BF16)
        nc.gpsimd.dma_start(onesend[:], onesend_d[:])
        startexp = const_pool.tile([K, 1], F32)
        nc.gpsimd.dma_start(startexp[:], startexp_d[:])

        e_sb = const_pool.tile([K, tot_cols], e_dt)
        out_sb = const_pool.tile([2, 2 * G * COLS_G], F32)
        # boundary-state snapshots (entry states at s == W-1)
        snap = [const_pool.tile([K, COLS_G], BF16, name=f"snap{g}")
                for g in range(G)]

        dma_eng = nc.gpsimd if DMA_GPSIMD else nc.sync

        def preload():
            # Preload all emissions to SBUF (few huge DMAs).
            pre_cols = PRE * G * COLS_G
            for c0 in range(0, tot_cols, pre_cols):
                c1 = min(tot_cols, c0 + pre_cols)
                dma_eng.dma_start(e_sb[:, c0:c1], emx[:, c0:c1])

        if not PRELOAD_IN_LOOP:
            preload()

        loop_cm = tc.For_i(0, repeat, 1) if repeat > 1 else None
        if loop_cm is not None:
            ctx.enter_context(loop_cm)

        if PRELOAD_IN_LOOP:
            preload()

        v = []
        for g in range(G):
            vg = state_pool.tile([K, COLS_G], BF16, name=f"v{g}")
            nc.vector.memset(vg[:], 1.0)
            v.append(vg)

        for s in range(1, NV + 1):
            for g in range(G):
                ps = psum_pool.tile([K, COLS_G], F32)
                for n0 in range(0, COLS_G, 512):
                    n1 = min(COLS_G, n0 + 512)
                    nc.tensor.matmul(ps[:, n0:n1], expT[:], v[g][:, n0:n1],
                                     start=True, stop=True)

                ecol = ((s - 1) * G + g) * COLS_G
                e_t = e_sb[:, ecol:ecol + COLS_G]

                vn = state_pool.tile([K, COLS_G], BF16, name=f"vn{g}")
                # ACT path: copy+cast cols [0:XS] to bf16, DVE multiplies
                # them at 2x; DVE path: cols [XS:] at 1x from PSUM.
                if XS > 0:
                    mid = mid_pool.tile([K, XS], BF16)
                    nc.scalar.copy(mid[:], ps[:, 0:XS])
                    nc.vector.tensor_mul(vn[:, 0:XS], mid[:], e_t[:, 0:XS])
                if XS < COLS_G:
                    nc.vector.tensor_mul(vn[:, XS:], ps[:, XS:], e_t[:, XS:])
                v[g] = vn

                if s == W and g == 0:
                    # chunk 0 exact init at t=0:
                    # u0 = exp(start) * exp(em0 - c)
                    nc.vector.tensor_scalar_mul(v[0][:, 0:BC], e_t[:, 0:BC],
                                                startexp[:])
                if s == W - 1:
                    # snapshot entry state (time = c*TC - 1); sums later
                    nc.vector.tensor_copy(snap[g][:], v[g][:])

        # Boundary sums, off the critical path:
        #   entries: 1^T snap ; finals: [1^T v ; exp(end)^T v]
        for g in range(G):
            bp = bsum_pool.tile([1, COLS_G], F32)
            for n0 in range(0, COLS_G, 512):
                n1 = min(COLS_G, n0 + 512)
                nc.tensor.matmul(bp[:, n0:n1], onesend[:, 0:1],
                                 snap[g][:, n0:n1], start=True, stop=True)
            nc.scalar.copy(out_sb[0:1, g * COLS_G:(g + 1) * COLS_G], bp[:])

            bpf = bsum_pool.tile([2, COLS_G], F32)
            for n0 in range(0, COLS_G, 512):
                n1 = min(COLS_G, n0 + 512)
                nc.tensor.matmul(bpf[:, n0:n1], onesend[:], v[g][:, n0:n1],
                                 start=True, stop=True)
            off = G * COLS_G + g * COLS_G
            nc.scalar.copy(out_sb[0:2, off:off + COLS_G], bpf[:])

        dma_eng.dma_start(sums[:], out_sb[:])

    nc.compile()
    return nc


def _host_prep(emissions, transitions, start_transitions, end_transitions):
    """Per-core input maps with host-precomputed exp and chunk layout.

    emx[k, ((s-1)*G + g)*COLS_G + l*BC + b]
      = exp(em[core*BC + b, tau, k] - CSHIFT),
      tau = clip((g*CG + l)*TC - W + s, 0, T-1).
    """
    if E8:
        e_full = np.exp(emissions.astype(np.float32) - C_E)
        e_full = e_full.astype(ml_dtypes.float8_e4m3)
    else:
        e_full = np.exp(emissions.astype(np.float32) - CSHIFT)
        e_full = e_full.astype(ml_dtypes.bfloat16)

    s_idx = np.arange(1, NV + 1)
    g_idx = np.arange(G)
    l_idx = np.arange(CG)
    # tau[s, g, l]
    tau = (g_idx[None, :, None] * CG + l_idx[None, None, :]) * TC \
        - W + s_idx[:, None, None]
    tau = np.clip(tau, 0, T - 1)

    wshift = (CSHIFT - C_E) if E8 else 0.0
    expt_in = np.ascontiguousarray(
        np.exp(transitions.astype(np.float32) - wshift)
        .astype(ml_dtypes.bfloat16))
    onesend_in = np.stack(
        [np.ones(K, np.float32),
         np.exp(end_transitions.astype(np.float32))], axis=1)
    onesend_in = np.ascontiguousarray(onesend_in.astype(ml_dtypes.bfloat16))
    startexp_in = np.ascontiguousarray(
        np.exp(start_transitions.astype(np.float32))[:, None])

    in_maps = []
    for core in range(N_CORES):
        ec = e_full[core * BC:(core + 1) * BC]              # [BC, T, K]
        eT = np.ascontiguousarray(ec.transpose(2, 1, 0))    # [K, T, BC]
        emx = eT[:, tau, :].reshape(K, NV * G * COLS_G)
        in_maps.append({
            "emx": np.ascontiguousarray(emx),
            "expt": expt_in,
            "onesend": onesend_in,
            "startexp": startexp_in,
        })
    return in_maps


def _gold_score(em, tags, mask, trans, start, end):
    em = em.astype(np.float64)
    mask = mask.astype(np.float64)
    tg = tags.astype(np.int64)
    score = start.astype(np.float64)[tg[:, 0]]
    emit = np.take_along_axis(em, tg[:, :, None], axis=2)[:, :, 0]
    score = score + (emit * mask).sum(axis=1)
    score = score + (trans.astype(np.float64)[tg[:, :-1], tg[:, 1:]]
                     * mask[:, 1:]).sum(axis=1)
    seq_ends = mask.astype(np.int64).sum(axis=1) - 1
    last = tg[np.arange(tg.shape[0]), seq_ends]
    score = score + end.astype(np.float64)[last]
    return score


def _host_logz_fallback(em, trans, start, end):
    """Exact f64 forward algorithm (only used if mask is not all-ones)."""
    em = em.astype(np.float64)
    la = start.astype(np.float64) + em[:, 0, :]
    tr = trans.astype(np.float64)
    for t in range(1, em.shape[1]):
        sc = tr[None] + la[:, :, None] + em[:, t, None, :]
        m = sc.max(axis=1, keepdims=True)
        la = np.squeeze(m, 1) + np.log(np.exp(sc - m).sum(axis=1))
    x = la + end[None].astype(np.float64)
    m = x.max(axis=1, keepdims=True)
    return np.squeeze(m, 1) + np.log(np.exp(x - m).sum(axis=1))


def _assemble_logz(results):
    """Telescoped per-chunk log-gains -> logz[B] (host, f64)."""
    logz = np.zeros(B)
    for core in range(N_CORES):
        r = np.asarray(results[core]["sums"], dtype=np.float64)
        acc = None
        for c in range(C):
            g, l = divmod(c, CG)
            sl = slice(g * COLS_G + l * BC, g * COLS_G + l * BC + BC)
            entry = r[0, sl]
            off = G * COLS_G
            sl2 = slice(off + g * COLS_G + l * BC,
                        off + g * COLS_G + l * BC + BC)
            end1 = r[0, sl2]
            endE = r[1, sl2]
            if c == 0:
                acc = np.log(end1).copy()         # exact absolute scale
            elif c < C - 1:
                acc += np.log(end1) - np.log(entry)
            else:
                acc += np.log(endE) - np.log(entry)
        corr = (CSHIFT - C_E) if E8 else 0.0
        logz[core * BC:(core + 1) * BC] = acc + T * CSHIFT - corr
    return logz


def kernel(emissions, tags, mask, transitions, start_transitions,
           end_transitions):
    global _NC_CACHE
    emissions = np.ascontiguousarray(np.asarray(emissions, dtype=np.float32))
    tags = np.asarray(tags)
    mask = np.asarray(mask)
    transitions = np.asarray(transitions, dtype=np.float32)
    start_transitions = np.asarray(start_transitions, dtype=np.float32)
    end_transitions = np.asarray(end_transitions, dtype=np.float32)

    score = _gold_score(emissions, tags, mask, transitions,
                        start_transitions, end_transitions)

    if not np.all(mask == 1):
        logz = _host_logz_fallback(emissions, transitions,
                                   start_transitions, end_transitions)
        return np.float32(-(score - logz).mean())

    if _NC_CACHE is None:
        _NC_CACHE = _build_program()
    nc = _NC_CACHE

    in_maps = _host_prep(emissions, transitions, start_transitions,
                         end_transitions)
    results = run_bass_kernel_spmd(nc, in_maps, list(range(N_CORES))).results
    logz = _assemble_logz(results)
    return np.float32(-(score - logz).mean())
